# revision 10
# baseline (speedup 1.0000x reference)
"""Dual-path self-attention (DPSA) kernel for 8 Trainium2 NeuronCores.

Reference computation (B=2, S=2048, D=1024, H=16, DK=64):
    Q_sem = X_sem @ Wq_sem + bq_sem   (per-head)
    K_sem = X_sem @ Wk_sem + bk_sem
    V     = X_sem @ Wv + bv
    Q_sal = X_sal @ Wq_sal + bq_sal
    K_sal = X_sal @ Wk_sal + bk_sal
    A = (Q_sem K_sem^T + gamma * Q_sal K_sal^T) / sqrt(DK)
    A = softmax(mask ? A : -1e9)
    out = (A V) @ Wo + bo

Sharding: batch x head-group. Core c handles batch c//4 and heads
[4*(c%4), 4*(c%4)+4). Each core computes its 4 heads' partial output
projection sum_h(O_h @ Wo[rows_h]); the host reduces the 4 partials per
batch and adds bo.

Device-side layout ("transposed attention"):
  - X^T [D, S] resident in SBUF (fp16); projections produce Q^T/K^T
    directly: psum[m=channels, n=seq] = Wchunk.T @ X^T chunk.
  - QcatT/KcatT per head: [128, S] with semantic rows 0:64, salience
    rows 64:128. One contract-128 matmul computes
    A^T = (Q_sem K_sem^T + gamma Q_sal K_sal^T)^T / sqrt(DK) fused at
    full PE efficiency (scales folded into Q-side weights on host).
  - exp on ScalarE (PSUM->SBUF, fp16 out); AV accumulation with
    lhsT = [V_h | ones] so the softmax denominator falls out as row 64
    of the AV psum for free.
  - Normalization delayed past AV: O_unnorm^T scaled by 1/denom (fast
    DVE reciprocal + stride-0 DMA partition-broadcast) before Wo.
"""

import numpy as np

B, S, D, H = 2, 2048, 1024, 16
DK = D // H  # 64
N_CORES = 8
HG = 4  # head groups (cores per batch)
HPC = 4  # heads per core
DKC = HPC * DK  # 256 channels per core
QCHUNK = 512
NQC = S // QCHUNK  # 4
NKT = S // 128  # 16 key tiles
VSLOT = DK + 1  # V columns per (ktile, head) slot incl. ones column

_cached = {}


def _build_nc(with_qk_bias, with_v_bias, with_mask):
    import concourse.bass as bass
    import concourse.tile as tile
    from concourse import bacc, mybir

    fp16 = mybir.dt.float16
    fp32 = mybir.dt.float32

    nc = bacc.Bacc(None)

    # ---- DRAM I/O (per-core shards) ----
    xt_sem_d = nc.dram_tensor("xt_sem", [D, S], fp16, kind="ExternalInput")
    xt_sal_d = nc.dram_tensor("xt_sal", [D, S], fp16, kind="ExternalInput")
    # weights pre-rearranged on host: w_r[p, c*C + j] = W[c*128 + p, j]
    wq_d = nc.dram_tensor("wq", [128, 8 * DKC], fp16, kind="ExternalInput")
    wk_d = nc.dram_tensor("wk", [128, 8 * DKC], fp16, kind="ExternalInput")
    wqs_d = nc.dram_tensor("wqs", [128, 8 * DKC], fp16, kind="ExternalInput")
    wks_d = nc.dram_tensor("wks", [128, 8 * DKC], fp16, kind="ExternalInput")
    wv_d = nc.dram_tensor("wv", [128, 8 * DKC], fp16, kind="ExternalInput")
    wo_d = nc.dram_tensor("wo", [128, 2 * D], fp16, kind="ExternalInput")
    if with_qk_bias:
        bqk_d = nc.dram_tensor("bqk", [1, 4 * DKC], fp16, kind="ExternalInput")
    if with_v_bias:
        bv_d = nc.dram_tensor("bv", [1, DKC], fp16, kind="ExternalInput")
    if with_mask:
        mb_d = nc.dram_tensor("mb", [S, S], fp16, kind="ExternalInput")
    out_d = nc.dram_tensor("out", [S, D], fp16, kind="ExternalOutput")

    # denominator slot for head h: (partition row, column offset).
    # matmul operands only allow base partitions {0, 32, 64}, so head 3
    # lives at row 0 in a second column region.
    def dloc(h):
        return (0, S) if h == 3 else (32 * h, 0)

    with tile.TileContext(nc) as tc:
        with tc.tile_pool(name="persist", bufs=1) as persist:
            qcat = persist.tile([128, HPC, S], fp16)
            kcat = persist.tile([128, HPC, S], fp16)
            v_sb = persist.tile([128, NKT, HPC, VSLOT], fp16)
            ones_sb = persist.tile([1, QCHUNK], fp16)
            onesT_sb = persist.tile([128, 64], fp32)
            wq_sb = persist.tile([128, 8 * DKC], fp16)
            wk_sb = persist.tile([128, 8 * DKC], fp16)
            wqs_sb = persist.tile([128, 8 * DKC], fp16)
            wks_sb = persist.tile([128, 8 * DKC], fp16)
            wv_sb = persist.tile([128, 8 * DKC], fp16)
            wo_sb = persist.tile([128, 2 * D], fp16)

            nc.vector.memset(ones_sb[:], 1.0)
            nc.vector.memset(onesT_sb[:], 1.0)
            nc.vector.memset(v_sb[:, :, :, DK : DK + 1], 1.0)

            nc.sync.dma_start(out=wqs_sb[:], in_=wqs_d[:])
            nc.sync.dma_start(out=wks_sb[:], in_=wks_d[:])
            nc.sync.dma_start(out=wq_sb[:], in_=wq_d[:])
            nc.sync.dma_start(out=wk_sb[:], in_=wk_d[:])
            nc.sync.dma_start(out=wv_sb[:], in_=wv_d[:])
            nc.sync.dma_start(out=wo_sb[:], in_=wo_d[:])
            if with_qk_bias:
                bqk_sb = persist.tile([1, 4 * DKC], fp16)
                nc.sync.dma_start(out=bqk_sb[:], in_=bqk_d[:])
            if with_v_bias:
                bv_sb = persist.tile([1, DKC], fp16)
                nc.sync.dma_start(out=bv_sb[:], in_=bv_d[:])
            if with_mask:
                mb_sb = persist.tile([128, NKT, S], fp16)
                nc.sync.dma_start(out=mb_sb[:], in_=mb_d.rearrange("(t p) s -> p t s", p=128))

            # ================= projections =================
            with tc.tile_pool(name="xtp", bufs=1) as xtp:
                xt_sal = xtp.tile([128, 8, S], fp16)
                xt_sem = xtp.tile([128, 8, S], fp16)
                nc.sync.dma_start(out=xt_sal[:], in_=xt_sal_d.rearrange("(c p) s -> p c s", p=128))
                nc.sync.dma_start(out=xt_sem[:], in_=xt_sem_d.rearrange("(c p) s -> p c s", p=128))

                projs = [
                    (wqs_sb, xt_sal, qcat, 64, 1),  # (W, X^T, dest, row0, bias-idx)
                    (wks_sb, xt_sal, kcat, 64, 3),
                    (wq_sb, xt_sem, qcat, 0, 0),
                    (wk_sb, xt_sem, kcat, 0, 2),
                ]
                with tc.tile_pool(name="ppsum", bufs=2, space="PSUM") as ppsum:
                    for w_sb, x_sb, dest, row0, bidx in projs:
                        for mt in range(2):  # head pair (2*mt, 2*mt+1)
                            for nh in range(2):  # seq halves of 1024
                                ps = ppsum.tile([128, 1024], fp32)
                                for nq in range(2):
                                    o_ap = ps[:, nq * 512 : (nq + 1) * 512]
                                    r_sl = slice(nh * 1024 + nq * 512, nh * 1024 + (nq + 1) * 512)
                                    for kc in range(8):
                                        nc.tensor.matmul(
                                            o_ap,
                                            w_sb[:, kc * DKC + mt * 128 : kc * DKC + (mt + 1) * 128],
                                            x_sb[:, kc, r_sl],
                                            start=(kc == 0),
                                            stop=(kc == 7 and not with_qk_bias),
                                        )
                                    if with_qk_bias:
                                        nc.tensor.matmul(
                                            o_ap,
                                            bqk_sb[:, bidx * DKC + mt * 128 : bidx * DKC + (mt + 1) * 128],
                                            ones_sb[:, :512],
                                            start=False,
                                            stop=True,
                                        )
                                s_sl = slice(nh * 1024, (nh + 1) * 1024)
                                nc.vector.tensor_copy(dest[row0 : row0 + 64, 2 * mt, s_sl], ps[0:64, :])
                                nc.scalar.copy(dest[row0 : row0 + 64, 2 * mt + 1, s_sl], ps[64:128, :])

                    # V: natural layout [s, dv]
                    for st in range(NKT):
                        ps = ppsum.tile([128, 1024], fp32)
                        vp = ps[:, :DKC]
                        for kc in range(8):
                            nc.tensor.matmul(
                                vp,
                                xt_sem[:, kc, st * 128 : (st + 1) * 128],
                                wv_sb[:, kc * DKC : (kc + 1) * DKC],
                                start=(kc == 0),
                                stop=(kc == 7 and not with_v_bias),
                            )
                        if with_v_bias:
                            nc.tensor.matmul(
                                vp, ones_sb[:, :128], bv_sb[:], start=False, stop=True
                            )
                        nc.vector.tensor_copy(
                            v_sb[:, st, :, 0:DK],
                            vp.rearrange("p (h d) -> p h d", h=HPC),
                        )

            # ================= attention / normalize / Wo =================
            with tc.tile_pool(name="late", bufs=1) as late:
                denom_sb = late.tile([128, 2 * S], fp32)
                recip_sb = late.tile([128, 2 * S], fp32)
                o_un = late.tile([64, HPC, S], fp16)
                ot = late.tile([128, 2, S], fp16)
                nc.vector.memset(denom_sb[:], 1.0)

                with (
                    tc.tile_pool(name="spsum", bufs=2, space="PSUM") as spsum,
                    tc.tile_pool(name="avpsum", bufs=2, space="PSUM") as avpsum,
                    tc.tile_pool(name="expp", bufs=3) as expp,
                ):
                    for h in range(HPC):
                        dr, dc = dloc(h)
                        for qc in range(NQC):
                            q_sl = slice(qc * QCHUNK, (qc + 1) * QCHUNK)
                            av = avpsum.tile([65, QCHUNK], fp32)
                            kt = 0
                            first = True
                            while kt < NKT:
                                ng = min(3, NKT - kt)
                                sp = spsum.tile([128, 3 * QCHUNK], fp32)
                                for g in range(ng):
                                    nc.tensor.matmul(
                                        sp[:, g * QCHUNK : (g + 1) * QCHUNK],
                                        kcat[:, h, (kt + g) * 128 : (kt + g + 1) * 128],
                                        qcat[:, h, q_sl],
                                        start=True,
                                        stop=True,
                                    )
                                if with_mask:
                                    for g in range(ng):
                                        nc.vector.tensor_tensor(
                                            sp[:, g * QCHUNK : (g + 1) * QCHUNK],
                                            sp[:, g * QCHUNK : (g + 1) * QCHUNK],
                                            mb_sb[:, kt + g, q_sl],
                                            mybir.AluOpType.add,
                                        )
                                et = expp.tile([128, 3 * QCHUNK], fp16)
                                nc.scalar.activation(
                                    et[:, : ng * QCHUNK],
                                    sp[:, : ng * QCHUNK],
                                    mybir.ActivationFunctionType.Exp,
                                )
                                for g in range(ng):
                                    nc.tensor.matmul(
                                        av,
                                        v_sb[:, kt + g, h, :],
                                        et[:, g * QCHUNK : (g + 1) * QCHUNK],
                                        start=first,
                                        stop=(kt + g == NKT - 1),
                                    )
                                    first = False
                                kt += ng
                            nc.vector.tensor_copy(o_un[:, h, q_sl], av[0:64, :])
                            nc.vector.tensor_copy(
                                denom_sb[dr : dr + 1, dc + qc * QCHUNK : dc + (qc + 1) * QCHUNK],
                                av[64:65, :],
                            )

                    nc.vector.reciprocal_approx_fast(out=recip_sb[:], in_=denom_sb[:])

                with (
                    tc.tile_pool(name="bcpsum", bufs=4, space="PSUM") as bcpsum,
                    tc.tile_pool(name="wopsum", bufs=2, space="PSUM") as wopsum,
                    tc.tile_pool(name="outp", bufs=3) as outp,
                ):
                    for h in range(HPC):
                        dr, dc = dloc(h)
                        for qc in range(NQC):
                            q_sl = slice(qc * QCHUNK, (qc + 1) * QCHUNK)
                            bc = bcpsum.tile([64, QCHUNK], fp32)
                            nc.tensor.matmul(
                                bc,
                                onesT_sb[dr : dr + 1, :],
                                recip_sb[dr : dr + 1, dc + qc * QCHUNK : dc + (qc + 1) * QCHUNK],
                                start=True,
                                stop=True,
                            )
                            nc.vector.tensor_tensor(
                                ot[(h % 2) * 64 : (h % 2) * 64 + 64, h // 2, q_sl],
                                o_un[:, h, q_sl],
                                bc[:],
                                mybir.AluOpType.mult,
                            )

                    for st in range(NKT):
                        wp = wopsum.tile([128, D], fp32)
                        for nh in range(2):
                            for cc in range(2):
                                nc.tensor.matmul(
                                    wp[:, nh * 512 : (nh + 1) * 512],
                                    ot[:, cc, st * 128 : (st + 1) * 128],
                                    wo_sb[:, cc * D + nh * 512 : cc * D + (nh + 1) * 512],
                                    start=(cc == 0),
                                    stop=(cc == 1),
                                )
                        ob = outp.tile([128, D], fp16)
                        if st % 2 == 0:
                            nc.vector.tensor_copy(ob[:], wp[:])
                        else:
                            nc.scalar.copy(ob[:], wp[:])
                        nc.sync.dma_start(out=out_d[st * 128 : (st + 1) * 128, :], in_=ob[:])

    nc.compile()
    return nc


def _get_nc(key):
    if key not in _cached:
        _cached[key] = _build_nc(*key)
    return _cached[key]


def _rearrange_w(w):
    # [1024, C] -> [128, 8*C] with w_r[p, c*C + j] = w[c*128 + p, j]
    C = w.shape[1]
    return np.ascontiguousarray(
        w.reshape(8, 128, C).transpose(1, 0, 2).reshape(128, 8 * C)
    )


def kernel(X_sem, X_sal, mask, Wq_sem, bq_sem, Wk_sem, bk_sem, Wv, bv,
           Wq_sal, bq_sal, Wk_sal, bk_sal, Wo, bo, gamma):
    from concourse.bass_utils import run_bass_kernel_spmd

    X_sem = np.asarray(X_sem)
    X_sal = np.asarray(X_sal)
    mask = np.asarray(mask)
    f32 = np.float32
    scale = f32(1.0 / np.sqrt(DK))
    g = f32(np.asarray(gamma).reshape(()))

    wq_full = (np.asarray(Wq_sem) * scale).astype(np.float16)
    bq_full = (np.asarray(bq_sem) * scale).astype(np.float16)
    wqs_full = (np.asarray(Wq_sal) * (g * scale)).astype(np.float16)
    bqs_full = (np.asarray(bq_sal) * (g * scale)).astype(np.float16)
    wk_full = np.asarray(Wk_sem).astype(np.float16)
    bk_full = np.asarray(bk_sem).astype(np.float16)
    wks_full = np.asarray(Wk_sal).astype(np.float16)
    bks_full = np.asarray(bk_sal).astype(np.float16)
    wv_full = np.asarray(Wv).astype(np.float16)
    bv_full = np.asarray(bv).astype(np.float16)
    wo_full = np.asarray(Wo).astype(np.float16)

    with_qk_bias = bool(
        np.any(np.asarray(bq_sem)) or np.any(np.asarray(bq_sal))
        or np.any(np.asarray(bk_sem)) or np.any(np.asarray(bk_sal))
    )
    with_v_bias = bool(np.any(np.asarray(bv)))
    with_mask = not bool(np.all(mask))

    nc = _get_nc((with_qk_bias, with_v_bias, with_mask))

    xt = []
    for b in range(B):
        xt.append((
            np.ascontiguousarray(X_sem[b].T.astype(np.float16)),
            np.ascontiguousarray(X_sal[b].T.astype(np.float16)),
        ))

    in_maps = []
    for c in range(N_CORES):
        b, hg = c // HG, c % HG
        blk = slice(hg * DKC, (hg + 1) * DKC)
        m = {
            "xt_sem": xt[b][0],
            "xt_sal": xt[b][1],
            "wq": _rearrange_w(wq_full[:, blk]),
            "wk": _rearrange_w(wk_full[:, blk]),
            "wqs": _rearrange_w(wqs_full[:, blk]),
            "wks": _rearrange_w(wks_full[:, blk]),
            "wv": _rearrange_w(wv_full[:, blk]),
            "wo": np.ascontiguousarray(
                wo_full[blk].reshape(2, 128, D).transpose(1, 0, 2).reshape(128, 2 * D)
            ),
        }
        if with_qk_bias:
            m["bqk"] = np.concatenate(
                [bq_full[blk], bqs_full[blk], bk_full[blk], bks_full[blk]]
            ).reshape(1, 4 * DKC)
        if with_v_bias:
            m["bv"] = bv_full[blk].reshape(1, DKC)
        if with_mask:
            mb = np.where(mask[b, 0] == 0, np.float16(-30000.0), np.float16(0.0))
            m["mb"] = np.ascontiguousarray(mb.T)
        in_maps.append(m)

    res = run_bass_kernel_spmd(nc, in_maps, core_ids=list(range(N_CORES)))

    out = np.zeros((B, S, D), dtype=f32)
    for c in range(N_CORES):
        out[c // HG] += res.results[c]["out"].astype(f32)
    out += np.asarray(bo).astype(f32)
    return out
